# revision 12
# baseline (speedup 1.0000x reference)
"""Encoder layer (pre-norm attention + MLP) on 8 Trainium2 cores.

Sharding: core = (batch b in 0..3, half hf in 0..1). Each core receives the
full 2048-token sequence of batch b, transposed to [E, S] and rolled so the
core's own 1024 tokens are columns 0:1024 (attention and LN are invariant to
key order, so rolling keeps the program identical across cores). The core
computes K/V over the full sequence and everything else only for its own
tokens. No collectives; the host reassembles the 8 shards.

Layouts on device are feature-major ([feature, token]) throughout so no
on-chip transposes are needed. LayerNorm gamma/beta are folded into the
following matmul weights on the host. Matmuls run as float32r (full PE rate,
fp32 storage).
"""

import numpy as np
from contextlib import ExitStack

import concourse.bacc as bacc
import concourse.mybir as mybir
import concourse.tile as tile
from concourse.bass_utils import run_bass_kernel_spmd

F32 = mybir.dt.float32
F32R = mybir.dt.float32r
AF = mybir.ActivationFunctionType
OP = mybir.AluOpType

B, S, E, H, D, FF = 4, 2048, 1024, 16, 64, 4096
TOWN = 1024  # tokens owned per core
ET = E // 128  # 8
FT = FF // 128  # 32
NCORES = 8
EPS = 1e-6


def _build():
    nc = bacc.Bacc()

    x_t = nc.dram_tensor("x_t", [E, S], F32R, kind="ExternalInput")
    wq_t = nc.dram_tensor("wq_t", [E, E], F32R, kind="ExternalInput")
    wk_t = nc.dram_tensor("wk_t", [E, E], F32R, kind="ExternalInput")
    wv_t = nc.dram_tensor("wv_t", [E, E], F32R, kind="ExternalInput")
    qb = nc.dram_tensor("qb", [E], F32, kind="ExternalInput")
    kb = nc.dram_tensor("kb", [E], F32, kind="ExternalInput")
    vb = nc.dram_tensor("vb", [E], F32R, kind="ExternalInput")
    wout_t = nc.dram_tensor("wout_t", [E, E], F32R, kind="ExternalInput")
    ob = nc.dram_tensor("ob", [E], F32, kind="ExternalInput")
    wfc1_t = nc.dram_tensor("wfc1_t", [E, FF], F32R, kind="ExternalInput")
    f1b = nc.dram_tensor("f1b", [FF], F32, kind="ExternalInput")
    wfc2_t = nc.dram_tensor("wfc2_t", [FF, E], F32R, kind="ExternalInput")

    x2_out = nc.dram_tensor("x2_out", [E, TOWN], F32, kind="ExternalOutput")
    mlp_out = nc.dram_tensor("mlp_out", [E, TOWN], F32, kind="ExternalOutput")

    with tile.TileContext(nc) as tc, ExitStack() as ctx:
        dram = ctx.enter_context(tc.tile_pool(name="dram", bufs=1, space="DRAM"))
        q_d = dram.tile([E, TOWN], F32R)
        k_d = dram.tile([E, S], F32R)
        v_d = dram.tile([S, H * 65], F32R)  # per head: 64 v cols + 1 ones col
        h_d = dram.tile([FF, TOWN], F32R)

        consts = ctx.enter_context(tc.tile_pool(name="consts", bufs=1))
        ones_f32 = consts.tile([128, 128], F32)
        nc.vector.memset(ones_f32, 1.0)
        ones128 = consts.tile([128, 128], F32R)
        nc.vector.tensor_copy(ones128, ones_f32)
        qb_sb = consts.tile([128, ET], F32)
        kb_sb = consts.tile([128, ET], F32)
        ob_sb = consts.tile([128, ET], F32)
        f1b_sb = consts.tile([128, FT], F32)
        nc.sync.dma_start(out=qb_sb, in_=qb.rearrange("(a p) -> p a", p=128))
        nc.sync.dma_start(out=kb_sb, in_=kb.rearrange("(a p) -> p a", p=128))
        nc.sync.dma_start(out=ob_sb, in_=ob.rearrange("(a p) -> p a", p=128))
        nc.sync.dma_start(out=f1b_sb, in_=f1b.rearrange("(a p) -> p a", p=128))
        # v bias broadcast across all partitions (v is token-major)
        vb_row = consts.tile([1, E], F32R)
        nc.sync.dma_start(out=vb_row, in_=vb[None, :])
        vb_bc = consts.tile([128, E], F32)
        with tc.tile_pool(name="stats_p", bufs=2, space="PSUM") as stats_p:
            for c in range(2):
                ps = stats_p.tile([128, 512], F32, tag="vbbc")
                nc.tensor.matmul(
                    ps, ones128[0:1, :],
                    vb_row[:, c * 512:(c + 1) * 512],
                    start=True, stop=True,
                )
                nc.scalar.activation(vb_bc[:, c * 512:(c + 1) * 512], ps,
                                     AF.Copy)

        # persistent stats rows (broadcast across partitions)
        mean1 = consts.tile([128, S], F32)
        rstd1 = consts.tile([128, S], F32)
        mean2 = consts.tile([128, TOWN], F32)
        rstd2 = consts.tile([128, TOWN], F32)

        inv_e = 1.0 / E
        unb = float(E) / (E - 1.0)  # E/(E-1) for unbiased variance

        def ln_stats(src_tiles, n_chunks, mean_bc, rstd_bc, tmp_pool, ps_pool):
            # src_tiles[c] is [128, ET, 512]; stats over the feature dim via
            # ones-matmul (result broadcast to all 128 partitions for free).
            for c in range(n_chunks):
                sl = slice(c * 512, (c + 1) * 512)
                ps_sum = ps_pool.tile([128, 512], F32, tag="ps_sum")
                ps_ssq = ps_pool.tile([128, 512], F32, tag="ps_ssq")
                for a in range(ET):
                    xa = src_tiles[c][:, a, :]
                    xsq = tmp_pool.tile([128, 512], F32R, tag="xsq")
                    nc.vector.tensor_tensor(xsq, xa, xa, OP.mult)
                    nc.tensor.matmul(ps_sum, ones128, xa,
                                     start=(a == 0), stop=(a == ET - 1))
                    nc.tensor.matmul(ps_ssq, ones128, xsq,
                                     start=(a == 0), stop=(a == ET - 1))
                m = tmp_pool.tile([128, 512], F32, tag="m")
                nc.vector.tensor_scalar_mul(m, ps_sum, inv_e)
                nc.vector.tensor_copy(mean_bc[:, sl], m)
                msq = tmp_pool.tile([128, 512], F32, tag="msq")
                nc.vector.tensor_tensor(msq, m, m, OP.mult)
                nc.vector.tensor_scalar_mul(msq, msq, unb)
                var = tmp_pool.tile([128, 512], F32, tag="var")
                nc.vector.tensor_scalar(var, ps_ssq, 1.0 / (E - 1.0), None,
                                        OP.mult)
                nc.vector.tensor_tensor(var, var, msq, OP.subtract)
                std = tmp_pool.tile([128, 512], F32, tag="std")
                nc.scalar.activation(std, var, AF.Sqrt)
                nc.vector.tensor_scalar_add(std, std, EPS)
                nc.vector.reciprocal(rstd_bc[:, sl], std)

        # ---------------- Stage A: LN1 stats + z1 over full sequence -------
        with tc.tile_pool(name="pz1", bufs=1) as pz1:
            z1 = pz1.tile([128, ET, S], F32R)
            with tc.tile_pool(name="pa", bufs=1) as pa, \
                 tc.tile_pool(name="pa_tmp", bufs=3) as pa_tmp, \
                 tc.tile_pool(name="pa_ps", bufs=2, space="PSUM") as pa_ps:
                x_c = []
                for c in range(4):
                    xc = pa.tile([128, ET, 512], F32R, tag=f"xc{c}")
                    nc.sync.dma_start(out=xc, in_=x_t.rearrange(
                        "(a p) s -> p a s", p=128)[:, :, c * 512:(c + 1) * 512])
                    x_c.append(xc)
                ln_stats(x_c, 4, mean1, rstd1, pa_tmp, pa_ps)
                for c in range(4):
                    sl = slice(c * 512, (c + 1) * 512)
                    for a in range(ET):
                        nc.vector.tensor_tensor(
                            z1[:, a, sl], x_c[c][:, a, :], mean1[:, sl],
                            OP.subtract)
                        nc.vector.tensor_tensor(
                            z1[:, a, sl], z1[:, a, sl], rstd1[:, sl], OP.mult)

            # ------------- Stage B: QKV projections → DRAM ----------------
            # q (own tokens only)
            with tc.tile_pool(name="pbq_w", bufs=2) as pbw, \
                 tc.tile_pool(name="pbq_ev", bufs=4) as pbe, \
                 tc.tile_pool(name="pbq_ps", bufs=4, space="PSUM") as pbp:
                for ot in range(ET):
                    w_ot = pbw.tile([128, ET, 128], F32R, tag="w")
                    nc.sync.dma_start(out=w_ot, in_=wq_t.rearrange(
                        "(a p) o -> p a o", p=128)[:, :, ot * 128:(ot + 1) * 128])
                    pss = [pbp.tile([128, 512], F32, tag="ps", name=f"psq{ot}_{i}") for i in range(2)]
                    for a in range(ET):
                        for c in range(2):
                            nc.tensor.matmul(
                                pss[c], w_ot[:, a, :],
                                z1[:, a, c * 512:(c + 1) * 512],
                                start=(a == 0), stop=(a == ET - 1))
                    for c in range(2):
                        ev = pbe.tile([128, 512], F32R, tag="ev")
                        nc.scalar.activation(ev, pss[c], AF.Identity,
                                             bias=qb_sb[:, ot:ot + 1])
                        nc.sync.dma_start(
                            out=q_d[ot * 128:(ot + 1) * 128,
                                    c * 512:(c + 1) * 512],
                            in_=ev)
            # k (full sequence)
            with tc.tile_pool(name="pbk_w", bufs=2) as pbw, \
                 tc.tile_pool(name="pbk_ev", bufs=6) as pbe, \
                 tc.tile_pool(name="pbk_ps", bufs=8, space="PSUM") as pbp:
                for ot in range(ET):
                    w_ot = pbw.tile([128, ET, 128], F32R, tag="w")
                    nc.sync.dma_start(out=w_ot, in_=wk_t.rearrange(
                        "(a p) o -> p a o", p=128)[:, :, ot * 128:(ot + 1) * 128])
                    pss = [pbp.tile([128, 512], F32, tag="ps", name=f"psk{ot}_{i}") for i in range(4)]
                    for a in range(ET):
                        for c in range(4):
                            nc.tensor.matmul(
                                pss[c], w_ot[:, a, :],
                                z1[:, a, c * 512:(c + 1) * 512],
                                start=(a == 0), stop=(a == ET - 1))
                    for c in range(4):
                        ev = pbe.tile([128, 512], F32R, tag="ev")
                        nc.scalar.activation(ev, pss[c], AF.Identity,
                                             bias=kb_sb[:, ot:ot + 1])
                        nc.sync.dma_start(
                            out=k_d[ot * 128:(ot + 1) * 128,
                                    c * 512:(c + 1) * 512],
                            in_=ev)
            # v (full sequence, token-major, +ones column per head)
            with tc.tile_pool(name="pbv_w", bufs=1) as pbw, \
                 tc.tile_pool(name="pbv_ev", bufs=4) as pbe, \
                 tc.tile_pool(name="pbv_ps", bufs=4, space="PSUM") as pbp:
                wv_sb = pbw.tile([128, ET, E], F32R)
                nc.sync.dma_start(out=wv_sb,
                                  in_=wv_t.rearrange("(a p) o -> p a o", p=128))
                v_d3 = v_d.rearrange("s (h w) -> s h w", w=65)
                for tt in range(S // 128):
                    tsl = slice(tt * 128, (tt + 1) * 128)
                    pss = [pbp.tile([128, 512], F32, tag="ps", name=f"psv{tt}_{i}") for i in range(2)]
                    for a in range(ET):
                        for c in range(2):
                            nc.tensor.matmul(
                                pss[c], z1[:, a, tsl],
                                wv_sb[:, a, c * 512:(c + 1) * 512],
                                start=(a == 0), stop=(a == ET - 1))
                    for c in range(2):
                        ev = pbe.tile([128, 512], F32R, tag="ev")
                        nc.vector.tensor_tensor(
                            ev, pss[c], vb_bc[:, c * 512:(c + 1) * 512], OP.add)
                        nc.sync.dma_start(
                            out=v_d3[tsl, c * 8:(c + 1) * 8, 0:64],
                            in_=ev.rearrange("p (h w) -> p h w", w=64))
                    nc.sync.dma_start(out=v_d3[tsl, :, 64],
                                      in_=ones128[:, 0:H])

        # ---------------- Stage C: attention per head ----------------------
        with tc.tile_pool(name="pctxn", bufs=1) as pctxn:
            ctxn = pctxn.tile([128, ET, TOWN], F32R)
            with tc.tile_pool(name="pc_in", bufs=2) as pcin, \
                 tc.tile_pool(name="pc_pr", bufs=4) as pcpr, \
                 tc.tile_pool(name="pc_misc", bufs=4) as pcm, \
                 tc.tile_pool(name="pc_pss", bufs=4, space="PSUM") as pss_p, \
                 tc.tile_pool(name="pc_psx", bufs=2, space="PSUM") as psx_p, \
                 tc.tile_pool(name="pc_psr", bufs=2, space="PSUM") as psr_p:
                v_d3 = v_d.rearrange("(n p) hw -> p n hw", p=128)
                for h in range(H):
                    kh = pcin.tile([64, S], F32R, tag="kh")
                    nc.sync.dma_start(out=kh, in_=k_d[h * 64:(h + 1) * 64, :])
                    qh = pcin.tile([64, TOWN], F32R, tag="qh")
                    nc.sync.dma_start(out=qh, in_=q_d[h * 64:(h + 1) * 64, :])
                    vh = pcin.tile([128, S // 128, 65], F32R, tag="vh")
                    nc.sync.dma_start(
                        out=vh, in_=v_d3[:, :, h * 65:(h + 1) * 65])
                    for qc in range(2):
                        qsl = slice(qc * 512, (qc + 1) * 512)
                        ctx_ps = psx_p.tile([65, 512], F32, tag="ctx")
                        for kt in range(S // 128):
                            s_ps = pss_p.tile([128, 512], F32, tag="s")
                            nc.tensor.matmul(
                                s_ps, kh[:, kt * 128:(kt + 1) * 128],
                                qh[:, qsl], start=True, stop=True)
                            pr = pcpr.tile([128, 512], F32R, tag="pr")
                            nc.scalar.activation(pr, s_ps, AF.Exp, scale=0.125)
                            nc.tensor.matmul(
                                ctx_ps, vh[:, kt, :], pr,
                                start=(kt == 0), stop=(kt == S // 128 - 1))
                        rec = pcm.tile([1, 512], F32R, tag="rec")
                        with nc.allow_low_precision(
                                reason="f32r rounding of softmax denom"):
                            nc.vector.reciprocal(rec, ctx_ps[64:65, :])
                        rb_ps = psr_p.tile([64, 512], F32, tag="rb")
                        nc.tensor.matmul(rb_ps, ones128[0:1, 0:64],
                                         rec, start=True, stop=True)
                        rb = pcm.tile([64, 512], F32, tag="rbs")
                        nc.scalar.activation(rb, rb_ps, AF.Copy)
                        p0 = (h % 2) * 64
                        nc.vector.tensor_tensor(
                            ctxn[p0:p0 + 64, h // 2, qsl],
                            ctx_ps[0:64, :], rb, OP.mult)

            # ------------- Stage D: out-proj + residual -------------------
            with tc.tile_pool(name="px2", bufs=1) as px2:
                x2 = px2.tile([128, ET, TOWN], F32)
                x2r = px2.tile([128, ET, TOWN], F32R)
                with tc.tile_pool(name="pd_in", bufs=1) as pdin, \
                     tc.tile_pool(name="pd_w", bufs=2) as pdw, \
                     tc.tile_pool(name="pd_ev", bufs=4) as pde, \
                     tc.tile_pool(name="pd_ps", bufs=4, space="PSUM") as pdp:
                    x_own = pdin.tile([128, ET, TOWN], F32R)
                    nc.sync.dma_start(out=x_own, in_=x_t.rearrange(
                        "(a p) s -> p a s", p=128)[:, :, 0:TOWN])
                    for ot in range(ET):
                        w_ot = pdw.tile([128, ET, 128], F32R, tag="w")
                        nc.sync.dma_start(out=w_ot, in_=wout_t.rearrange(
                            "(a p) o -> p a o", p=128)[:, :,
                                                       ot * 128:(ot + 1) * 128])
                        pss = [pdp.tile([128, 512], F32, tag="ps",
                                       name=f"psd{ot}_{i}") for i in range(2)]
                        for a in range(ET):
                            for c in range(2):
                                nc.tensor.matmul(
                                    pss[c], w_ot[:, a, :],
                                    ctxn[:, a, c * 512:(c + 1) * 512],
                                    start=(a == 0), stop=(a == ET - 1))
                        for c in range(2):
                            sl = slice(c * 512, (c + 1) * 512)
                            ev = pde.tile([128, 512], F32, tag="ev")
                            nc.scalar.activation(ev, pss[c], AF.Identity,
                                                 bias=ob_sb[:, ot:ot + 1])
                            nc.vector.tensor_tensor(
                                x2[:, ot, sl], ev, x_own[:, ot, sl], OP.add)
                            nc.vector.tensor_tensor(
                                x2r[:, ot, sl], ev, x_own[:, ot, sl], OP.add)
                        nc.sync.dma_start(
                            out=x2_out[ot * 128:(ot + 1) * 128, :],
                            in_=x2[:, ot, :])

                # --------- Stage E: LN2 stats + z2 (own tokens) -----------
                with tc.tile_pool(name="pz2", bufs=1) as pz2:
                    z2 = pz2.tile([128, ET, TOWN], F32R)
                    with tc.tile_pool(name="pe_tmp", bufs=3) as pe_tmp, \
                         tc.tile_pool(name="pe_ps", bufs=2,
                                      space="PSUM") as pe_ps:
                        x2v = [x2r[:, :, c * 512:(c + 1) * 512]
                               for c in range(2)]
                        ln_stats(x2v, 2, mean2, rstd2, pe_tmp, pe_ps)
                        for c in range(2):
                            sl = slice(c * 512, (c + 1) * 512)
                            for a in range(ET):
                                nc.vector.tensor_tensor(
                                    z2[:, a, sl], x2[:, a, sl], mean2[:, sl],
                                    OP.subtract)
                                nc.vector.tensor_tensor(
                                    z2[:, a, sl], z2[:, a, sl], rstd2[:, sl],
                                    OP.mult)

                    # --------- Stage F: fc1 + gelu → DRAM -----------------
                    with tc.tile_pool(name="pf_w", bufs=2) as pfw, \
                         tc.tile_pool(name="pf_ev", bufs=4) as pfe, \
                         tc.tile_pool(name="pf_ps", bufs=4,
                                      space="PSUM") as pfp:
                        for ft in range(FT):
                            w_ft = pfw.tile([128, ET, 128], F32R, tag="w")
                            nc.sync.dma_start(out=w_ft, in_=wfc1_t.rearrange(
                                "(a p) o -> p a o", p=128)[
                                    :, :, ft * 128:(ft + 1) * 128])
                            pss = [pfp.tile([128, 512], F32, tag="ps",
                                           name=f"psf{ft}_{i}") for i in range(2)]
                            for a in range(ET):
                                for c in range(2):
                                    nc.tensor.matmul(
                                        pss[c], w_ft[:, a, :],
                                        z2[:, a, c * 512:(c + 1) * 512],
                                        start=(a == 0), stop=(a == ET - 1))
                            for c in range(2):
                                ev = pfe.tile([128, 512], F32R, tag="ev")
                                nc.scalar.activation(
                                    ev, pss[c], AF.Gelu,
                                    bias=f1b_sb[:, ft:ft + 1])
                                nc.sync.dma_start(
                                    out=h_d[ft * 128:(ft + 1) * 128,
                                            c * 512:(c + 1) * 512],
                                    in_=ev)

        # ---------------- Stage G: fc2 → mlp_out (transposed) --------------
        with tc.tile_pool(name="pg_h", bufs=1) as pgh, \
             tc.tile_pool(name="pg_w", bufs=2) as pgw, \
             tc.tile_pool(name="pg_ev", bufs=4) as pge, \
             tc.tile_pool(name="pg_ps", bufs=2, space="PSUM") as pgp:
            for half in range(2):
                hsl = slice(half * 512, (half + 1) * 512)
                hg = pgh.tile([128, FT, 512], F32R, tag="hg")
                nc.sync.dma_start(out=hg, in_=h_d.rearrange(
                    "(a p) s -> p a s", p=128)[:, :, hsl])
                for ot in range(ET):
                    w_ot = pgw.tile([128, FT, 128], F32R, tag="w")
                    nc.sync.dma_start(out=w_ot, in_=wfc2_t.rearrange(
                        "(a p) o -> p a o", p=128)[:, :,
                                                   ot * 128:(ot + 1) * 128])
                    ps = pgp.tile([128, 512], F32, tag="ps")
                    for f in range(FT):
                        nc.tensor.matmul(ps, w_ot[:, f, :],
                                         hg[:, f, :],
                                         start=(f == 0), stop=(f == FT - 1))
                    ev = pge.tile([128, 512], F32, tag="ev")
                    nc.scalar.activation(ev, ps, AF.Copy)
                    nc.sync.dma_start(
                        out=mlp_out[ot * 128:(ot + 1) * 128, hsl], in_=ev)

    nc.finalize()
    return nc


_NC_CACHE = {}


def _get_nc():
    if "nc" not in _NC_CACHE:
        _NC_CACHE["nc"] = _build()
    return _NC_CACHE["nc"]


def _prepare_in_maps(inputs):
    f = np.float32
    x = np.asarray(inputs["x"], f)
    w_qkv = np.asarray(inputs["w_qkv"], np.float64)
    ln1_w = np.asarray(inputs["ln1_w"], np.float64)
    ln1_b = np.asarray(inputs["ln1_b"], np.float64)
    ln2_w = np.asarray(inputs["ln2_w"], np.float64)
    ln2_b = np.asarray(inputs["ln2_b"], np.float64)
    w_fc1 = np.asarray(inputs["w_fc1"], np.float64)

    wqkv_s = (w_qkv * ln1_w[None, :])  # fold LN1 gamma
    qkv_bias = ln1_b @ np.asarray(inputs["w_qkv"], np.float64).T  # [3E]
    wqkv_t = np.ascontiguousarray(wqkv_s.T, f)  # [E, 3E]
    wq_t = np.ascontiguousarray(wqkv_t[:, 0:E])
    wk_t = np.ascontiguousarray(wqkv_t[:, E:2 * E])
    wv_t = np.ascontiguousarray(wqkv_t[:, 2 * E:3 * E])
    qb = np.ascontiguousarray(qkv_bias[0:E], f)
    kb = np.ascontiguousarray(qkv_bias[E:2 * E], f)
    vb = np.ascontiguousarray(qkv_bias[2 * E:3 * E], f)

    wout_t = np.ascontiguousarray(np.asarray(inputs["w_out"], f).T)
    ob = np.ascontiguousarray(np.asarray(inputs["b_out"], f))

    wfc1_s = (w_fc1 * ln2_w[None, :])
    f1b = np.ascontiguousarray(
        np.asarray(inputs["b_fc1"], np.float64) + ln2_b @ w_fc1.T, f)
    wfc1_t = np.ascontiguousarray(wfc1_s.T, f)  # [E, F]
    wfc2_t = np.ascontiguousarray(np.asarray(inputs["w_fc2"], f).T)  # [F, E]

    shared = dict(wq_t=wq_t, wk_t=wk_t, wv_t=wv_t, qb=qb, kb=kb, vb=vb,
                  wout_t=wout_t, ob=ob, wfc1_t=wfc1_t, f1b=f1b,
                  wfc2_t=wfc2_t)
    in_maps = []
    for core in range(NCORES):
        b, hf = divmod(core, 2)
        xs = np.roll(x[b], -hf * TOWN, axis=0)  # own tokens first
        x_t = np.ascontiguousarray(xs.T)  # [E, S]
        in_maps.append(dict(x_t=x_t, **shared))
    return in_maps


def _assemble(inputs, results):
    f = np.float32
    b_fc2 = np.asarray(inputs["b_fc2"], f)
    out = np.empty((B, S, E), f)
    for core in range(NCORES):
        b, hf = divmod(core, 2)
        r = results[core]
        shard = r["x2_out"].T + r["mlp_out"].T + b_fc2[None, :]
        out[b, hf * TOWN:(hf + 1) * TOWN, :] = shard
    return out


def run(inputs, **spmd_kwargs):
    nc = _get_nc()
    in_maps = _prepare_in_maps(inputs)
    res = run_bass_kernel_spmd(nc, in_maps, core_ids=list(range(NCORES)),
                               **spmd_kwargs)
    return _assemble(inputs, res.results), res


def kernel(**inputs):
    out, _ = run(inputs)
    return out
